# revision 9
# baseline (speedup 1.0000x reference)
"""Trainium2 Bass kernel for nn_CausalAttention (RMSNorm + QKV + rotary +
causal attention with KV cache + out-projection).

Sharding: 8 cores = 4 batches x 2 head-groups (8 heads each).
Per core the device computes, for its (batch, head-group):
  - RMSNorm(x_b) -> xn (token-major), PE-transposed to xnT (feature-major)
  - q,k feature-major + v token-major projections (bf16 matmuls, f32 psum)
  - rotary on q and k (cache k loaded pre-transposed from host)
  - scoresT = k_rot^T-chunks @ q_rot  ([j,i] orientation), exp on ACT engine
    (softmax max-subtraction is skipped: scores are O(6) so exp is safe)
  - softmax denominator via ones-matmul (broadcast across partitions)
  - attn-outT = v-chunks^T @ expT, normalized by reciprocal of the denominator
  - partial out = attn-outT^T-chunks @ w_out rows (token-major) -> HBM
Host: shards inputs, folds norm_w into w_qkv, builds cos/sin tables and causal
mask tiles, then sums the per-pair partial outputs and assembles cached_kv.
"""

import numpy as np
import ml_dtypes

import concourse.bacc as bacc
import concourse.mybir as mybir
from concourse.tile import TileContext
from concourse.bass_utils import run_bass_kernel_spmd

F32 = mybir.dt.float32
BF16 = mybir.dt.bfloat16
AF = mybir.ActivationFunctionType

NPBF16 = ml_dtypes.bfloat16

B = 4
N = 1024          # new tokens
CL = 1024         # cache length
J = 2048          # total keys
DIM = 2048
HEADS = 16
DH = 128
HL = 8            # heads per core
P = 128
KC = 16           # dim chunks of 128
NT = 8            # new-token tiles of 128
JC = 16           # key chunks of 128
IT = 2            # query tiles of 512
SCALE = float(DH) ** -0.5
EPS = float(np.finfo(np.float32).eps)

_NC_CACHE = {}


def _build_nc():
    nc = bacc.Bacc("TRN2", target_bir_lowering=False, num_devices=8)

    x = nc.dram_tensor("x", (N, DIM), F32, kind="ExternalInput")
    wqk = nc.dram_tensor("wqk", (16, KC, P, P), BF16, kind="ExternalInput")
    wv = nc.dram_tensor("wv", (2, KC, P, 512), BF16, kind="ExternalInput")
    wout = nc.dram_tensor("wout", (8, 4, P, 512), BF16, kind="ExternalInput")
    ckT = nc.dram_tensor("ckT", (HL, P, CL), F32, kind="ExternalInput")
    cv = nc.dram_tensor("cv", (HL, CL, DH), F32, kind="ExternalInput")
    cosT = nc.dram_tensor("cosT", (P, J), BF16, kind="ExternalInput")
    sinT = nc.dram_tensor("sinT", (P, J), BF16, kind="ExternalInput")
    maskT = nc.dram_tensor("maskT", (4, P, 512), BF16, kind="ExternalInput")
    id16 = nc.dram_tensor("id16", (P, P), BF16, kind="ExternalInput")
    id32 = nc.dram_tensor("id32", (P, P), F32, kind="ExternalInput")
    ones = nc.dram_tensor("ones", (P, P), BF16, kind="ExternalInput")

    pout = nc.dram_tensor("pout", (N, DIM), F32, kind="ExternalOutput")
    knew = nc.dram_tensor("knew", (HL, N, DH), F32, kind="ExternalOutput")
    vnew = nc.dram_tensor("vnew", (HL, N, DH), F32, kind="ExternalOutput")

    with TileContext(nc) as tc:
        with (
            tc.tile_pool(name="pers", bufs=1) as pers,
            tc.tile_pool(name="persAB", bufs=1) as pab,
        ):
            # constants
            mask_sb = pers.tile([P, 4 * 512], BF16, tag="mask", name="mask")
            for a in range(4):
                nc.sync.dma_start(out=mask_sb[:, a * 512:(a + 1) * 512],
                                  in_=maskT[a])
            ones_sb = pers.tile([P, P], BF16, tag="ones", name="ones")
            nc.sync.dma_start(out=ones_sb, in_=ones[:, :])

            # persistent activations (live through phase A+B)
            qrot = [pab.tile([P, N], BF16, tag=f"qrot{h}", name=f"qrot{h}") for h in range(HL)]
            kTrot = [pab.tile([P, J], BF16, tag=f"kTrot{h}", name=f"kTrot{h}") for h in range(HL)]
            vt = [pab.tile([P, HL * DH], BF16, tag=f"vt{j}", name=f"vt{j}") for j in range(JC)]
            aoT = [pers.tile([P, N], BF16, tag=f"aoT{h}", name=f"aoT{h}") for h in range(HL)]

            def rotary(dst, src, pos0, L):
                """dst[128, L] (bf16) = rotary(src[128, L] f32-ish feature-major),
                positions pos0..pos0+L. Uses cos_sb/sin_sb slices."""
                sw = rotp.tile([P, L], F32, tag="rotsw", name="rotsw")
                nc.vector.tensor_copy(sw[0:64, :], src[64:128, :])
                nc.vector.tensor_copy(sw[64:128, :], src[0:64, :])
                m = rotp.tile([P, L], F32, tag="rotm", name="rotm")
                nc.vector.tensor_mul(m, src, cos_sb[:, pos0:pos0 + L])
                nc.vector.tensor_mul(sw, sw, sin_sb[:, pos0:pos0 + L])
                nc.vector.tensor_add(dst, m, sw)

            # ---------------- Phase A: norm, QKV, rotary, cache load ---------
            with (
                tc.tile_pool(name="pA", bufs=2) as pA,
                tc.tile_pool(name="pAw", bufs=4) as pAw,
                tc.tile_pool(name="pAs", bufs=1) as pAs,
                tc.tile_pool(name="rotp", bufs=1) as rotp,
                tc.tile_pool(name="psA", bufs=2, space="PSUM") as psA,
                tc.tile_pool(name="psQK", bufs=2, space="PSUM") as psQK,
            ):
                cos_sb = pAs.tile([P, J], BF16, tag="cos", name="cos")
                sin_sb = pAs.tile([P, J], BF16, tag="sin", name="sin")
                nc.sync.dma_start(out=cos_sb, in_=cosT[:, :])
                nc.sync.dma_start(out=sin_sb, in_=sinT[:, :])
                id16_sb = pAs.tile([P, P], BF16, tag="id16", name="id16")
                id32_sb = pAs.tile([P, P], F32, tag="id32", name="id32")
                nc.sync.dma_start(out=id16_sb, in_=id16[:, :])
                nc.sync.dma_start(out=id32_sb, in_=id32[:, :])

                # cache: v -> vt tiles (bf16), k -> rotary -> kTrot[:, 0:CL]
                for h in range(HL):
                    ckst = pA.tile([P, CL], F32, tag="ckst", name="ckst", bufs=1)
                    nc.sync.dma_start(out=ckst, in_=ckT[h])
                    rotary(kTrot[h][:, 0:CL], ckst, 0, CL)
                for h in range(HL):
                    for tj in range(8):
                        cvst = pAw.tile([P, DH], F32, tag="cvst", name="cvst")
                        nc.sync.dma_start(
                            out=cvst, in_=cv[h, tj * P:(tj + 1) * P, :])
                        nc.vector.tensor_copy(
                            vt[tj][:, h * DH:(h + 1) * DH], cvst)

                # RMSNorm + transpose -> xnT
                xnT = [pAs.tile([P, N], BF16, tag=f"xnT{k}", name=f"xnT{k}") for k in range(KC)]
                for ti in range(NT):
                    xa = [pA.tile([P, 1024], F32, tag=f"xa{hh}", name=f"xa{hh}")
                          for hh in range(2)]
                    r0 = ti * P
                    nc.sync.dma_start(out=xa[0], in_=x[r0:r0 + P, 0:1024])
                    nc.sync.dma_start(out=xa[1], in_=x[r0:r0 + P, 1024:2048])
                    sq = pA.tile([P, 1024], BF16, tag="sqscr", name="sqscr", bufs=1)
                    ss = [pA.tile([P, 1], F32, tag=f"ss{hh}", name=f"ss{hh}") for hh in range(2)]
                    nc.scalar.activation(sq, xa[0], AF.Square, accum_out=ss[0])
                    nc.scalar.activation(sq, xa[1], AF.Square, accum_out=ss[1])
                    sse = pA.tile([P, 1], F32, tag="sse", name="sse")
                    nc.vector.scalar_tensor_tensor(
                        out=sse, in0=ss[0], scalar=1.0, in1=ss[1],
                        op0=mybir.AluOpType.mult, op1=mybir.AluOpType.add)
                    # mean + eps, then rsqrt = sqrt(1/x)
                    rr = pA.tile([P, 1], F32, tag="rr", name="rr")
                    nc.vector.tensor_scalar(
                        out=rr, in0=sse, scalar1=1.0 / DIM, scalar2=EPS,
                        op0=mybir.AluOpType.mult, op1=mybir.AluOpType.add)
                    ri = pA.tile([P, 1], F32, tag="ri", name="ri")
                    nc.vector.reciprocal(ri, rr)
                    s = pA.tile([P, 1], F32, tag="s", name="s")
                    nc.scalar.activation(s, ri, AF.Sqrt)
                    xn = [pA.tile([P, 1024], BF16, tag=f"xn{hh}", name=f"xn{hh}", bufs=1)
                          for hh in range(2)]
                    nc.scalar.activation(xn[0], xa[0], AF.Copy, scale=s)
                    nc.scalar.activation(xn[1], xa[1], AF.Copy, scale=s)
                    for kc in range(KC):
                        src = xn[kc // 8][:, (kc % 8) * P:(kc % 8 + 1) * P]
                        tp = psA.tile([P, P], BF16, tag="tps", name="tpose")
                        nc.tensor.transpose(tp, src, id16_sb)
                        nc.vector.tensor_copy(
                            xnT[kc][:, ti * P:(ti + 1) * P], tp)

                # q and k projections (feature-major out)
                for fc in range(16):
                    psq = [psQK.tile([P, 512], F32, tag="psqk", name="psqk",
                                    bufs=4)
                           for _ in range(IT)]
                    for kc in range(KC):
                        w = pAw.tile([P, P], BF16, tag="wqk", name="wqk")
                        nc.sync.dma_start(out=w, in_=wqk[fc, kc])
                        for it in range(IT):
                            nc.tensor.matmul(
                                psq[it], w, xnT[kc][:, it * 512:(it + 1) * 512],
                                start=(kc == 0), stop=(kc == KC - 1))
                    if fc < 8:
                        h = fc
                        for it in range(IT):
                            rotary(qrot[h][:, it * 512:(it + 1) * 512],
                                   psq[it], CL + it * 512, 512)
                    else:
                        h = fc - 8
                        knT = pA.tile([P, N], F32, tag="knT", name="knT", bufs=1)
                        for it in range(IT):
                            nc.vector.tensor_copy(
                                knT[:, it * 512:(it + 1) * 512], psq[it])
                        rotary(kTrot[h][:, CL:J], knT, CL, N)
                        for tb in range(NT):
                            tp32 = psA.tile([P, P], F32, tag="tps", name="tp32")
                            nc.tensor.transpose(
                                tp32, knT[:, tb * P:(tb + 1) * P], id32_sb)
                            kst = pAw.tile([P, P], F32, tag="kst", name="kst")
                            nc.scalar.copy(kst, tp32)
                            nc.sync.dma_start(
                                out=knew[h, tb * P:(tb + 1) * P, :], in_=kst)

                # v projection (token-major out)
                for fg in range(2):
                    wv_sb = pAs.tile([P, KC, 512], BF16, tag="wv", name="wv")
                    for kc in range(KC):
                        nc.sync.dma_start(out=wv_sb[:, kc], in_=wv[fg, kc])
                    for ti in range(NT):
                        psv = psQK.tile([P, 512], F32, tag="psv", name="psv")
                        for kc in range(KC):
                            nc.tensor.matmul(
                                psv, xnT[kc][:, ti * P:(ti + 1) * P],
                                wv_sb[:, kc],
                                start=(kc == 0), stop=(kc == KC - 1))
                        vst = pA.tile([P, 512], F32, tag="vst", name="vst", bufs=1)
                        nc.scalar.copy(vst, psv)
                        for sub in range(4):
                            h = fg * 4 + sub
                            nc.sync.dma_start(
                                out=vnew[h, ti * P:(ti + 1) * P, :],
                                in_=vst[:, sub * P:(sub + 1) * P])
                        nc.vector.tensor_copy(
                            vt[8 + ti][:, fg * 512:(fg + 1) * 512], psv)

            # ---------------- Phase B: attention --------------------------
            with (
                tc.tile_pool(name="pB", bufs=3) as pB,
                tc.tile_pool(name="pBr", bufs=2) as pBr,
                tc.tile_pool(name="psSC", bufs=2, space="PSUM") as psSC,
                tc.tile_pool(name="psS", bufs=2, space="PSUM") as psS,
                tc.tile_pool(name="psAO", bufs=2, space="PSUM") as psAO,
            ):
                for h in range(HL):
                    for it in range(IT):
                        jn = 12 if it == 0 else 16
                        pss = psS.tile([P, 512], F32, tag="pss", name="pss")
                        pao = psAO.tile([P, 512], F32, tag="pao", name="pao")
                        q_sl = qrot[h][:, it * 512:(it + 1) * 512]
                        for jc in range(jn):
                            sc = psSC.tile([P, 512], F32, tag="sc", name="sc")
                            nc.tensor.matmul(
                                sc, kTrot[h][:, jc * P:(jc + 1) * P], q_sl,
                                start=True, stop=True)
                            ex = pB.tile([P, 512], BF16, tag="ex", name="ex")
                            nc.scalar.activation(ex, sc, AF.Exp, scale=SCALE)
                            a = jc - (8 if it == 0 else 12)
                            if 0 <= a < 4:
                                nc.vector.tensor_mul(
                                    ex, ex, mask_sb[:, a * 512:(a + 1) * 512])
                            nc.tensor.matmul(pss, ones_sb, ex,
                                             start=(jc == 0), stop=(jc == jn - 1))
                            nc.tensor.matmul(
                                pao, vt[jc][:, h * DH:(h + 1) * DH], ex,
                                start=(jc == 0), stop=(jc == jn - 1))
                        rec = pBr.tile([P, 512], F32, tag="rec", name="rec")
                        nc.vector.reciprocal(rec, pss)
                        nc.vector.tensor_mul(
                            aoT[h][:, it * 512:(it + 1) * 512], pao, rec)

            # ---------------- Phase C: out projection ----------------------
            with (
                tc.tile_pool(name="pC", bufs=1) as pC,
                tc.tile_pool(name="psC", bufs=2, space="PSUM") as psC,
            ):
                wout_sb = pC.tile([P, 32, 512], BF16, tag="wout", name="wout")
                for fc in range(8):
                    for og in range(4):
                        nc.sync.dma_start(out=wout_sb[:, fc * 4 + og],
                                          in_=wout[fc, og])
                for ic in range(8):
                    po = [psC.tile([P, 512], F32, tag=f"po{og}", name=f"po{og}") for og in range(4)]
                    for fc in range(8):
                        lhs = aoT[fc][:, ic * P:(ic + 1) * P]
                        for og in range(4):
                            nc.tensor.matmul(po[og], lhs,
                                             wout_sb[:, fc * 4 + og],
                                             start=(fc == 0), stop=(fc == 7))
                    for og in range(4):
                        post = pC.tile([P, 512], F32, tag="post", name="post",
                                       bufs=6)
                        nc.scalar.copy(post, po[og])
                        nc.sync.dma_start(
                            out=pout[ic * P:(ic + 1) * P,
                                     og * 512:(og + 1) * 512],
                            in_=post)

    nc.compile()
    return nc


def _get_nc():
    if "nc" not in _NC_CACHE:
        _NC_CACHE["nc"] = _build_nc()
    return _NC_CACHE["nc"]


def _prep_inputs(x, cache, rotary_emb, context_mask, norm_w, w_qkv, w_out):
    """Build the 8 per-core input maps (host-side sharding)."""
    x = np.asarray(x, dtype=np.float32)
    cache = np.asarray(cache, dtype=np.float32)
    rotary_emb = np.asarray(rotary_emb, dtype=np.float32)
    norm_w = np.asarray(norm_w, dtype=np.float32)
    w_qkv = np.asarray(w_qkv, dtype=np.float32)
    w_out = np.asarray(w_out, dtype=np.float32)

    w_eff = (norm_w[:, None] * w_qkv)

    # rotary tables, feature-major, sign-folded sin
    cos = np.cos(rotary_emb).T.astype(NPBF16)            # (128, J)
    sinT = np.sin(rotary_emb).T
    sinTs = np.concatenate([-sinT[0:64], sinT[64:128]], axis=0).astype(NPBF16)

    # multiplicative causal mask tiles (bf16 0/1), M[a][j, i] = j+128a <= i
    jj = np.arange(P)[:, None]
    ii = np.arange(512)[None, :]
    maskT = np.stack([(jj + P * a <= ii) for a in range(4)]).astype(NPBF16)

    id16 = np.eye(P, dtype=NPBF16)
    id32 = np.eye(P, dtype=np.float32)
    ones = np.ones((P, P), dtype=NPBF16)

    per_hg = {}
    for hg in range(2):
        qs = hg * 1024
        wq = w_eff[:, qs:qs + 1024]
        wk = w_eff[:, 2048 + qs:2048 + qs + 1024]
        wv_ = w_eff[:, 4096 + qs:4096 + qs + 1024]
        wqk_ = np.concatenate([wq, wk], axis=1).astype(NPBF16)   # (2048, 2048)
        # (fc, kc, 128, 128)
        wqk_c = np.ascontiguousarray(
            wqk_.reshape(KC, P, 16, P).transpose(2, 0, 1, 3))
        # (fg, kc, 128, 512)
        wv_c = np.ascontiguousarray(
            wv_.astype(NPBF16).reshape(KC, P, 2, 512).transpose(2, 0, 1, 3))
        # (fc, og, 128, 512)
        wo = w_out[qs:qs + 1024, :].astype(NPBF16)
        wo_c = np.ascontiguousarray(
            wo.reshape(8, P, 4, 512).transpose(0, 2, 1, 3))
        per_hg[hg] = (wqk_c, wv_c, wo_c)

    in_maps = []
    for c in range(8):
        b, hg = c // 2, c % 2
        hs = slice(hg * HL, (hg + 1) * HL)
        wqk_c, wv_c, wo_c = per_hg[hg]
        ckT_ = np.ascontiguousarray(cache[b, 0, hs].transpose(0, 2, 1))
        cv_ = np.ascontiguousarray(cache[b, 1, hs])
        in_maps.append({
            "x": np.ascontiguousarray(x[b]),
            "wqk": wqk_c, "wv": wv_c, "wout": wo_c,
            "ckT": ckT_, "cv": cv_,
            "cosT": cos, "sinT": sinTs, "maskT": maskT,
            "id16": id16, "id32": id32, "ones": ones,
        })
    return in_maps


def kernel(x, cache, rotary_emb, context_mask, norm_w, w_qkv, w_out):
    cache = np.asarray(cache, dtype=np.float32)
    in_maps = _prep_inputs(x, cache, rotary_emb, context_mask,
                           norm_w, w_qkv, w_out)
    nc = _get_nc()
    res = run_bass_kernel_spmd(nc, in_maps, core_ids=list(range(8))).results

    out = np.empty((B, N, DIM), dtype=np.float32)
    cached_kv = np.empty((B, 2, HEADS, J, DH), dtype=np.float32)
    cached_kv[:, :, :, :CL, :] = cache
    for c in range(8):
        b, hg = c // 2, c % 2
        hs = slice(hg * HL, (hg + 1) * HL)
        if hg == 0:
            out[b] = res[2 * b]["pout"]
            out[b] += res[2 * b + 1]["pout"]
        cached_kv[b, 0, hs, CL:, :] = res[c]["knew"]
        cached_kv[b, 1, hs, CL:, :] = res[c]["vnew"]
    return out, cached_kv
